# revision 23
# baseline (speedup 1.0000x reference)
"""Trainium2 Bass kernel for the 2-bit-DoReFa quantized BasicBlock.

  out = conv3x3(q(bn2(conv3x3(q(bn1(x)), Wq1))), Wq2) + x
  q(h) = round(3*clip(relu(h),0,1))/3,  Wq = DoReFa-2bit(w) in {-1,-1/3,1/3,1}

Sharding: data-parallel over batch, 4 images per NeuronCore x 8 cores;
conv weights and BN parameters replicated.

Per-core kernel design:
  * Quantized activations/weights are exact small integers when scaled by 3:
    a3 in {0..3}, w3 in {-3,-1,1,3}.  Both are exact in fp8e4, so each 3x3
    conv becomes 9 accumulating DoubleRow 128x(2x128) matmuls per output
    tile with *exact* integer accumulation in fp32 PSUM.  The 1/9 rescale
    folds into the next stage's BN scale / final residual add.
  * The quant chain replicates the reference fp32 op sequence exactly
    (BN on DVE, Relu on ACT, min*3 and the 2^23 round-to-nearest-even
    trick on DVE), so the result bit-matches jnp's op-for-op math.
  * Padded activations live in a row-interleaved layout: row-slot
    s = 2*row + cin_half, each slot 57 wide (left pad col shared as the
    previous slot's right pad).  Every engine AP over this layout is a
    tight linear interval, so the Tile overlap tracker derives exact
    chunk-granular dependencies (a conv tile waits only on the quant of
    the rows it reads, not the whole image; costs ~5ns/matmul in 57B
    rhs read segments vs a plane layout, measured).
  * Input DMA: two chunk-serialized chains (one per cin half) plus a
    weight chain, so image 0's first rows, w1, and w2 all land just in
    time for their first consumers; later images stream behind.
  * conv2 tiles interleave one tile behind conv1 within the same image;
    stage-1 quant units of the next image spread between tile groups;
    conv2 lags two tiles so its last-row taps never wait on the
    freshest conv1 quant.
  * Short PE warmup burst gated on the (tiny) BN-param DMA primes the
    HAM clock gate just before the first real matmul.
"""
import os
from contextlib import ExitStack

import numpy as np

import bass_rust
import concourse.bacc as bacc
import concourse.tile as tile
from concourse import mybir
from concourse.bass_utils import run_bass_kernel_spmd

F32 = mybir.dt.float32
OP = mybir.AluOpType
MAGIC = 8388608.0  # 2**23

N_CORES = 8
N_IMG = 4
C = 256
H = W = 56
PW = W + 1                      # 57: left pad + 56 image cols
NPIX = H * W
RT = 8
NT = H // RT                    # 7 row tiles
TQ = RT * PW                    # 456
SLOT = 2 * PW                   # 114: one padded row, both cin halves
NPAD = SLOT * (H + 2) + 60      # 6672 incl. garbage-read slack
N_CHUNK = 4
CR = H // N_CHUNK               # 14 rows per quant chunk / DMA sub
ACT_DT = mybir.dt.float8e4
N_WARMUP = 8

LAST_EXEC_NS = None          # set when BASS_TRACE=1
_CACHED = {}


def _build():
    nc = bacc.Bacc("TRN2", target_bir_lowering=False, debug=False)

    x_d = nc.dram_tensor("x", [N_IMG, C, H, W], F32, kind="ExternalInput")
    w1_d = nc.dram_tensor("w1t", [128, 4608], ACT_DT, kind="ExternalInput")
    w2_d = nc.dram_tensor("w2t", [128, 4608], ACT_DT, kind="ExternalInput")
    prm_d = nc.dram_tensor("prm", [128, 8], F32, kind="ExternalInput")
    out_d = nc.dram_tensor("out", [N_IMG, C, H, W], F32, kind="ExternalOutput")

    xr = x_d.ap().rearrange("n (b k) h w -> n k b (h w)", b=2)
    outr = out_d.ap().rearrange("n (b k) h w -> n k b (h w)", b=2)

    with tile.TileContext(nc) as tc, ExitStack() as ctx:
        wpool = ctx.enter_context(tc.tile_pool(name="wpool", bufs=1))
        xpool = ctx.enter_context(tc.tile_pool(name="xpool", bufs=4))
        aqpool = ctx.enter_context(tc.tile_pool(name="aqpool", bufs=1))
        t1pool = ctx.enter_context(tc.tile_pool(name="t1pool", bufs=3))
        t2pool = ctx.enter_context(tc.tile_pool(name="t2pool", bufs=6))
        pspool = ctx.enter_context(tc.tile_pool(name="pspool", bufs=7,
                                                space="PSUM"))

        # Warmup source: memset SBUF, no DMA dependency.
        wsrc = wpool.tile([128, 584], ACT_DT)
        nc.gpsimd.memset(wsrc[:], 0.0)

        prm = wpool.tile([128, 8], F32)
        prm_dma = nc.sync.dma_start(prm[:], prm_d.ap())

        x_sbs = [None] * N_IMG
        xchain = {}

        def x_chunk_dma(img, ch, b):
            """One 14-row cin-half chunk (3136B/partition descriptors;
            smaller subs are descriptor-rate-bound at ~40GB/s/ring),
            chained per half so emission order == priority order."""
            sl = slice(ch * CR * W, (ch + 1) * CR * W)
            inst = nc.sync.dma_start(x_sbs[img][:, b, sl], xr[img][:, b, sl])
            if b in xchain:
                tile.add_dep_helper(inst.ins, xchain[b].ins, sync=True,
                                    reason="x chunk priority chain")
            xchain[b] = inst

        # Image 0 chunk 0 first: it gates the first conv tile.
        x_sbs[0] = xpool.tile([128, 2, NPIX], F32, tag="x", name="x_0")
        x_chunk_dma(0, 0, 0)
        x_chunk_dma(0, 0, 1)

        # Weights on one chain: w1 whole, then w2 in two links so
        # conv2 t0's first taps wait only on the first w2 half.
        w1_sb = wpool.tile([128, 4608], ACT_DT)
        w2_sb = wpool.tile([128, 4608], ACT_DT)
        wprev = nc.sync.dma_start(w1_sb[:], w1_d.ap())
        for sl in (slice(0, 1536), slice(1536, 4608)):
            inst = nc.sync.dma_start(w2_sb[:, sl], w2_d.ap()[:, sl])
            tile.add_dep_helper(inst.ins, wprev.ins, sync=True,
                                reason="w chain link")
            wprev = inst

        # Rest of image 0, then images 1-3, streamed behind on the chains.
        for ch in range(1, N_CHUNK):
            x_chunk_dma(0, ch, 0)
            x_chunk_dma(0, ch, 1)
        for img in range(1, N_IMG):
            x_sbs[img] = xpool.tile([128, 2, NPIX], F32, tag="x",
                                    name=f"x_{img}")
            for ch in range(N_CHUNK):
                x_chunk_dma(img, ch, 0)
                x_chunk_dma(img, ch, 1)

        # Fixed ping-pong padded activation buffers; borders zeroed once
        # (interior writes never touch them, so they stay zero on reuse).
        aq1s, aq2s = [], []
        for i in range(2):
            a1 = aqpool.tile([128, NPAD], ACT_DT, name=f"aq1_{i}", tag=f"aq1_{i}")
            a2 = aqpool.tile([128, NPAD], ACT_DT, name=f"aq2_{i}", tag=f"aq2_{i}")
            aq1s.append(a1)
            aq2s.append(a2)
            for a in (a1, a2):
                nc.gpsimd.memset(a[:, 0:SLOT], 0.0)             # top pad row
                nc.gpsimd.memset(a[:, SLOT * (H + 1):NPAD], 0.0)  # bottom+slack
                nc.gpsimd.memset(a[:, 0:SLOT * (H + 1)].rearrange(
                    "p (s c) -> p s c", c=PW)[:, :, 0:1], 0.0)  # left pads

        def qdst_ap(aq, blk, y0, rows):
            base = (2 * (y0 + 1) + blk) * PW
            return aq[:, base:base + rows * SLOT].rearrange(
                "p (r c) -> p r c", c=SLOT)[:, :, 1:1 + W]

        def rhs_ap(aq, t, ky, kx):
            base = 2 * (RT * t + ky) * PW + kx
            return aq[:, base:base + RT * SLOT].rearrange(
                "p (r j c) -> p j r c", j=2, c=PW)

        def quant_stage(src_ap, aq, blk, inv_col, bias_col, tmp_pool, rows,
                        y0, tag="qtmp", dve_only=False):
            """Exact replica of the reference fp32 op sequence:
            t=x*inv+b; relu; min(.,1)*3; round-to-nearest-even; cast.
            dve_only folds relu into a DVE (max 0, min 1) pass — same
            values, no cross-engine hop (lower latency)."""
            t = tmp_pool.tile([128, rows * W], F32, tag=tag)
            t3 = t[:].rearrange("p (r c) -> p r c", c=W)
            nc.vector.tensor_scalar(t3, src_ap, prm[:, inv_col:inv_col + 1],
                                    prm[:, bias_col:bias_col + 1], OP.mult, OP.add)
            if dve_only:
                nc.vector.tensor_scalar(t3, t3, 0.0, 1.0, OP.max, OP.min)
                nc.vector.tensor_scalar(t3, t3, 3.0, MAGIC, OP.mult, OP.add)
                dst3 = qdst_ap(aq, blk, y0, rows)
                nc.vector.tensor_scalar(dst3, t3, MAGIC, MAGIC,
                                        OP.subtract, OP.bypass)
                return
            nc.scalar.activation(t3, t3, mybir.ActivationFunctionType.Relu)
            nc.vector.tensor_scalar(t3, t3, 1.0, 3.0, OP.min, OP.mult)
            dst3 = qdst_ap(aq, blk, y0, rows)
            nc.vector.tensor_scalar(dst3, t3, MAGIC, MAGIC, OP.add, OP.subtract)

        def conv_tile(aq, w_sb, t, cb):
            ps = pspool.tile([128, TQ], F32, tag="ps")
            w4 = w_sb[:].rearrange("p (t j m) -> p t j m", t=9, j=2)
            for tap in range(9):
                ky, kx = divmod(tap, 3)
                lhsT = w4[:, tap, :, cb * 128:cb * 128 + 128]
                nc.tensor.matmul(ps[:], lhsT, rhs_ap(aq, t, ky, kx),
                                 perf_mode=mybir.MatmulPerfMode.DoubleRow,
                                 start=(tap == 0), stop=(tap == 8))
            return ps

        def stage1_units(img):
            """Quant thunks for image img, one per (chunk, blk)."""
            aq1 = aq1s[img % 2]
            x_sb = x_sbs[img]

            def make(ch, blk):
                def run():
                    sl = slice(ch * CR * W, (ch + 1) * CR * W)
                    src = x_sb[:, blk, sl].rearrange("p (r c) -> p r c", c=W)
                    quant_stage(src, aq1, blk, 0 + blk, 2 + blk, t1pool,
                                CR, ch * CR)
                return run
            return [make(ch, blk) for ch in range(N_CHUNK) for blk in range(2)]

        def conv1_tile_half(hh, cb):
            # image 0, tile 0 split into 4-row halves: the first half only
            # needs the first 7 quantized rows, starting the PE ~1.5us
            # earlier.  Full-size PSUM tag keeps the pool layout uniform.
            aq1, aq2 = aq1s[0], aq2s[0]
            ps = pspool.tile([128, TQ], F32, tag="ps")
            w4 = w1_sb[:].rearrange("p (t j m) -> p t j m", t=9, j=2)
            for tap in range(9):
                ky, kx = divmod(tap, 3)
                lhsT = w4[:, tap, :, cb * 128:cb * 128 + 128]
                base = 2 * (4 * hh + ky) * PW + kx
                rhs = aq1[:, base:base + 4 * SLOT].rearrange(
                    "p (r j c) -> p j r c", j=2, c=PW)
                nc.tensor.matmul(ps[:, 0:4 * PW], lhsT, rhs,
                                 perf_mode=mybir.MatmulPerfMode.DoubleRow,
                                 start=(tap == 0), stop=(tap == 8))
            psv = ps[:, 0:4 * PW].rearrange("p (r c) -> p r c",
                                            c=PW)[:, :, 0:W]
            quant_stage(psv, aq2, cb, 4 + cb, 6 + cb, t2pool, 4, 4 * hh,
                        tag="q2half")

        def conv1_tile(img, t, cb):
            aq1, aq2 = aq1s[img % 2], aq2s[img % 2]
            ps = conv_tile(aq1, w1_sb, t, cb)
            psv = ps[:].rearrange("p (r c) -> p r c", c=PW)[:, :, 0:W]
            quant_stage(psv, aq2, cb, 4 + cb, 6 + cb, t2pool, RT, t * RT)

        def conv2_tile(img, t, cb):
            aq2, x_sb = aq2s[img % 2], x_sbs[img]
            ps = conv_tile(aq2, w2_sb, t, cb)
            psv = ps[:].rearrange("p (r c) -> p r c", c=PW)[:, :, 0:W]
            res = x_sb[:, cb, t * RT * W: (t + 1) * RT * W]
            res3 = res.rearrange("p (r c) -> p r c", c=W)
            nc.vector.scalar_tensor_tensor(res3, psv, 1.0 / 9.0, res3,
                                           OP.mult, OP.add)
            nc.sync.dma_start(outr[img][:, cb, t * RT * W:
                                        (t + 1) * RT * W], res)

        # Image 0: chunk 0 quantizes in 7-row halves to cut head latency.
        for hh in range(2):
            for blk in range(2):
                src0 = x_sbs[0][:, blk, hh * 7 * W:(hh + 1) * 7 * W].rearrange(
                    "p (r c) -> p r c", c=W)
                quant_stage(src0, aq1s[0], blk, 0 + blk, 2 + blk, t1pool,
                            7, hh * 7, tag="qhalf")
        for f in stage1_units(0)[2:]:
            f()

        # PE warmup gated on the prm DMA; the PE sequencer itself is not
        # ready before ~10us, so a short burst suffices to prime HAM.
        wu_ps = pspool.tile([128, TQ], F32, tag="ps")
        for i in range(N_WARMUP):
            wu = nc.tensor.matmul(wu_ps[:], wsrc[:, 0:128], wsrc[:, 128:584],
                                  start=(i == 0), stop=(i == N_WARMUP - 1))
            if i == 0:
                tile.add_dep_helper(wu.ins, prm_dma.ins, sync=True,
                                    reason="gate warmup start on prm land")

        # Per-image: conv2 interleaves one tile behind conv1; next image's
        # stage-1 quant units spread between tile groups.
        for img in range(N_IMG):
            inter = stage1_units(img + 1) if img + 1 < N_IMG else []
            for t in range(NT):
                if img == 0 and t == 0:
                    for hh in (0, 1):
                        conv1_tile_half(hh, 0)
                        conv1_tile_half(hh, 1)
                else:
                    conv1_tile(img, t, 0)
                    conv1_tile(img, t, 1)
                if t > 1:
                    conv2_tile(img, t - 2, 0)
                    conv2_tile(img, t - 2, 1)
                if t >= 3:
                    # pop late: earlier pops head-of-line-block the DVE
                    # FIFO ahead of the q2 ops conv2 t0-t2 wait on
                    for _ in range(2):
                        if inter:
                            inter.pop(0)()
            for t in (NT - 2, NT - 1):
                conv2_tile(img, t, 0)
                conv2_tile(img, t, 1)
            for f in inter:
                f()

    nc.compile()
    return nc


def _host_prep(w1, w2, g1, b1, m1, v1, g2, b2, m2, v2):
    """BN folds + DoReFa weight quantization, replicating the reference's
    fp32 op sequence exactly (jax CPU), then weight layout transforms."""
    import jax
    import jax.numpy as jnp
    import ml_dtypes

    cpu = jax.local_devices(backend="cpu")[0]
    with jax.default_device(cpu):
        eps = jnp.float32(1e-5)
        inv1 = g1 / jnp.sqrt(v1 + eps)
        bias1 = b1 - m1 * inv1
        inv2 = g2 / jnp.sqrt(v2 + eps)
        bias2 = b2 - m2 * inv2
        inv2_9 = inv2 / np.float32(9.0)

        def wq3(w):
            wt = jnp.tanh(w)
            wn = wt / (2.0 * jnp.max(jnp.abs(wt))) + 0.5
            return 2.0 * jnp.round(wn * 3.0) - 3.0   # exact ints {-3,-1,1,3}

        wq1 = np.asarray(wq3(jnp.asarray(w1)), dtype=np.float32)
        wq2 = np.asarray(wq3(jnp.asarray(w2)), dtype=np.float32)
        inv1, bias1, inv2_9, bias2 = (
            np.asarray(a, dtype=np.float32)
            for a in (inv1, bias1, inv2_9, bias2))

    def wlayout(wq):
        # [cout, cin, ky, kx] -> [k(128), tap(9), blk(2), cout(256)]
        a = wq.reshape(256, 2, 128, 9)                     # cout, blk, k, tap
        return np.ascontiguousarray(np.transpose(a, (2, 3, 1, 0))
                                    .reshape(128, 4608)
                                    ).astype(ml_dtypes.float8_e4m3)

    prm = np.zeros((128, 8), np.float32)
    for col, v in enumerate((inv1, bias1)):
        prm[:, 2 * col] = v[0:128]
        prm[:, 2 * col + 1] = v[128:256]
    for col, v in enumerate((inv2_9, bias2)):
        prm[:, 4 + 2 * col] = v[0:128]
        prm[:, 4 + 2 * col + 1] = v[128:256]

    return {"w1t": wlayout(wq1), "w2t": wlayout(wq2), "prm": prm}


def kernel(x, w1, w2, g1, b1, m1, v1, g2, b2, m2, v2):
    global LAST_EXEC_NS
    x = np.asarray(x, dtype=np.float32)

    if "nc" not in _CACHED:
        _CACHED["nc"] = _build()
    nc = _CACHED["nc"]

    shared = _host_prep(w1, w2, g1, b1, m1, v1, g2, b2, m2, v2)
    in_maps = []
    for c in range(N_CORES):
        m = dict(shared)
        m["x"] = x[N_IMG * c:N_IMG * (c + 1)]
        in_maps.append(m)

    trace = bool(int(os.environ.get("BASS_TRACE", "0")))
    res = run_bass_kernel_spmd(nc, in_maps, core_ids=list(range(N_CORES)),
                               trace=trace)
    LAST_EXEC_NS = res.exec_time_ns
    return np.concatenate([res.results[c]["out"] for c in range(N_CORES)],
                          axis=0)


# revision 25
# speedup vs baseline: 1.0376x; 1.0376x over previous
"""Trainium2 Bass kernel for the 2-bit-DoReFa quantized BasicBlock.

  out = conv3x3(q(bn2(conv3x3(q(bn1(x)), Wq1))), Wq2) + x
  q(h) = round(3*clip(relu(h),0,1))/3,  Wq = DoReFa-2bit(w) in {-1,-1/3,1/3,1}

Sharding: data-parallel over batch, 4 images per NeuronCore x 8 cores;
conv weights and BN parameters replicated.

Per-core kernel design:
  * Quantized activations/weights are exact small integers when scaled by 3:
    a3 in {0..3}, w3 in {-3,-1,1,3}.  Both are exact in fp8e4, so each 3x3
    conv becomes 9 accumulating DoubleRow 128x(2x128) matmuls per output
    tile with *exact* integer accumulation in fp32 PSUM.  The 1/9 rescale
    folds into the next stage's BN scale / final residual add.
  * The quant chain replicates the reference fp32 op sequence exactly
    (BN on DVE, Relu on ACT, min*3 and the 2^23 round-to-nearest-even
    trick on DVE), so the result bit-matches jnp's op-for-op math.
  * Padded activations live in a row-interleaved layout: row-slot
    s = 2*row + cin_half, each slot 57 wide (left pad col shared as the
    previous slot's right pad).  Every engine AP over this layout is a
    tight linear interval, so the Tile overlap tracker derives exact
    chunk-granular dependencies (a conv tile waits only on the quant of
    the rows it reads, not the whole image; costs ~5ns/matmul in 57B
    rhs read segments vs a plane layout, measured).
  * Input DMA: two chunk-serialized chains (one per cin half) plus a
    weight chain, so image 0's first rows, w1, and w2 all land just in
    time for their first consumers; later images stream behind.
  * conv2 tiles interleave one tile behind conv1 within the same image;
    stage-1 quant units of the next image spread between tile groups;
    conv2 lags two tiles so its last-row taps never wait on the
    freshest conv1 quant.
  * Short PE warmup burst gated on the (tiny) BN-param DMA primes the
    HAM clock gate just before the first real matmul.
"""
import os
from contextlib import ExitStack

import numpy as np

import bass_rust
import concourse.bacc as bacc
import concourse.tile as tile
from concourse import mybir
from concourse.bass_utils import run_bass_kernel_spmd

F32 = mybir.dt.float32
OP = mybir.AluOpType
MAGIC = 8388608.0  # 2**23

N_CORES = 8
N_IMG = 4
C = 256
H = W = 56
PW = W + 1                      # 57: left pad + 56 image cols
NPIX = H * W
RT = 8
NT = H // RT                    # 7 row tiles
TQ = RT * PW                    # 456
SLOT = 2 * PW                   # 114: one padded row, both cin halves
NPAD = SLOT * (H + 2) + 60      # 6672 incl. garbage-read slack
N_CHUNK = 4
CR = H // N_CHUNK               # 14 rows per quant chunk / DMA sub
ACT_DT = mybir.dt.float8e4
N_WARMUP = 8

LAST_EXEC_NS = None          # set when BASS_TRACE=1
_CACHED = {}


def _build():
    nc = bacc.Bacc("TRN2", target_bir_lowering=False, debug=False)

    x_d = nc.dram_tensor("x", [N_IMG, C, H, W], F32, kind="ExternalInput")
    w1_d = nc.dram_tensor("w1t", [128, 4608], ACT_DT, kind="ExternalInput")
    w2_d = nc.dram_tensor("w2t", [128, 4608], ACT_DT, kind="ExternalInput")
    prm_d = nc.dram_tensor("prm", [128, 8], F32, kind="ExternalInput")
    out_d = nc.dram_tensor("out", [N_IMG, C, H, W], F32, kind="ExternalOutput")

    xr = x_d.ap().rearrange("n (b k) h w -> n k b (h w)", b=2)
    outr = out_d.ap().rearrange("n (b k) h w -> n k b (h w)", b=2)

    with tile.TileContext(nc) as tc, ExitStack() as ctx:
        wpool = ctx.enter_context(tc.tile_pool(name="wpool", bufs=1))
        xpool = ctx.enter_context(tc.tile_pool(name="xpool", bufs=4))
        aqpool = ctx.enter_context(tc.tile_pool(name="aqpool", bufs=1))
        t1pool = ctx.enter_context(tc.tile_pool(name="t1pool", bufs=3))
        t2pool = ctx.enter_context(tc.tile_pool(name="t2pool", bufs=6))
        pspool = ctx.enter_context(tc.tile_pool(name="pspool", bufs=7,
                                                space="PSUM"))

        # Warmup source: memset SBUF, no DMA dependency.
        wsrc = wpool.tile([128, 584], ACT_DT)
        nc.gpsimd.memset(wsrc[:], 0.0)

        prm = wpool.tile([128, 8], F32)
        prm_dma = nc.sync.dma_start(prm[:], prm_d.ap())

        x_sbs = [None] * N_IMG
        xchain = {}

        def x_chunk_dma(img, ch, b):
            """One 14-row cin-half chunk (3136B/partition descriptors;
            smaller subs are descriptor-rate-bound at ~40GB/s/ring),
            chained per half so emission order == priority order."""
            sl = slice(ch * CR * W, (ch + 1) * CR * W)
            inst = nc.sync.dma_start(x_sbs[img][:, b, sl], xr[img][:, b, sl])
            if b in xchain:
                tile.add_dep_helper(inst.ins, xchain[b].ins, sync=True,
                                    reason="x chunk priority chain")
            xchain[b] = inst

        # Image 0 chunk 0 first: it gates the first conv tile.
        x_sbs[0] = xpool.tile([128, 2, NPIX], F32, tag="x", name="x_0")
        x_chunk_dma(0, 0, 0)
        x_chunk_dma(0, 0, 1)

        # Weights on their own chain: w1 whole, then w2 behind it.
        w1_sb = wpool.tile([128, 4608], ACT_DT)
        w2_sb = wpool.tile([128, 4608], ACT_DT)
        w1_dma = nc.sync.dma_start(w1_sb[:], w1_d.ap())
        w2_dma = nc.sync.dma_start(w2_sb[:], w2_d.ap())
        tile.add_dep_helper(w2_dma.ins, w1_dma.ins, sync=True,
                            reason="w2 after w1")

        # Rest of image 0, then images 1-3, streamed behind on the chains.
        for ch in range(1, N_CHUNK):
            x_chunk_dma(0, ch, 0)
            x_chunk_dma(0, ch, 1)
        for img in range(1, N_IMG):
            x_sbs[img] = xpool.tile([128, 2, NPIX], F32, tag="x",
                                    name=f"x_{img}")
            for ch in range(N_CHUNK):
                x_chunk_dma(img, ch, 0)
                x_chunk_dma(img, ch, 1)

        # Fixed ping-pong padded activation buffers; borders zeroed once
        # (interior writes never touch them, so they stay zero on reuse).
        aq1s, aq2s = [], []
        for i in range(2):
            a1 = aqpool.tile([128, NPAD], ACT_DT, name=f"aq1_{i}", tag=f"aq1_{i}")
            a2 = aqpool.tile([128, NPAD], ACT_DT, name=f"aq2_{i}", tag=f"aq2_{i}")
            aq1s.append(a1)
            aq2s.append(a2)
            for a in (a1, a2):
                nc.gpsimd.memset(a[:, 0:SLOT], 0.0)             # top pad row
                nc.gpsimd.memset(a[:, SLOT * (H + 1):NPAD], 0.0)  # bottom+slack
                nc.gpsimd.memset(a[:, 0:SLOT * (H + 1)].rearrange(
                    "p (s c) -> p s c", c=PW)[:, :, 0:1], 0.0)  # left pads

        def qdst_ap(aq, blk, y0, rows):
            base = (2 * (y0 + 1) + blk) * PW
            return aq[:, base:base + rows * SLOT].rearrange(
                "p (r c) -> p r c", c=SLOT)[:, :, 1:1 + W]

        def rhs_ap(aq, t, ky, kx):
            base = 2 * (RT * t + ky) * PW + kx
            return aq[:, base:base + RT * SLOT].rearrange(
                "p (r j c) -> p j r c", j=2, c=PW)

        def quant_stage(src_ap, aq, blk, inv_col, bias_col, tmp_pool, rows,
                        y0, tag="qtmp", dve_only=False):
            """Exact replica of the reference fp32 op sequence:
            t=x*inv+b; relu; min(.,1)*3; round-to-nearest-even; cast.
            dve_only folds relu into a DVE (max 0, min 1) pass — same
            values, no cross-engine hop (lower latency)."""
            t = tmp_pool.tile([128, rows * W], F32, tag=tag)
            t3 = t[:].rearrange("p (r c) -> p r c", c=W)
            nc.vector.tensor_scalar(t3, src_ap, prm[:, inv_col:inv_col + 1],
                                    prm[:, bias_col:bias_col + 1], OP.mult, OP.add)
            if dve_only:
                nc.vector.tensor_scalar(t3, t3, 0.0, 1.0, OP.max, OP.min)
                nc.vector.tensor_scalar(t3, t3, 3.0, MAGIC, OP.mult, OP.add)
                dst3 = qdst_ap(aq, blk, y0, rows)
                nc.vector.tensor_scalar(dst3, t3, MAGIC, MAGIC,
                                        OP.subtract, OP.bypass)
                return
            nc.scalar.activation(t3, t3, mybir.ActivationFunctionType.Relu)
            nc.vector.tensor_scalar(t3, t3, 1.0, 3.0, OP.min, OP.mult)
            dst3 = qdst_ap(aq, blk, y0, rows)
            nc.vector.tensor_scalar(dst3, t3, MAGIC, MAGIC, OP.add, OP.subtract)

        def conv_tile(aq, w_sb, t, cb):
            ps = pspool.tile([128, TQ], F32, tag="ps")
            w4 = w_sb[:].rearrange("p (t j m) -> p t j m", t=9, j=2)
            for tap in range(9):
                ky, kx = divmod(tap, 3)
                lhsT = w4[:, tap, :, cb * 128:cb * 128 + 128]
                nc.tensor.matmul(ps[:], lhsT, rhs_ap(aq, t, ky, kx),
                                 perf_mode=mybir.MatmulPerfMode.DoubleRow,
                                 start=(tap == 0), stop=(tap == 8))
            return ps

        def stage1_units(img):
            """Quant thunks for image img, one per (chunk, blk)."""
            aq1 = aq1s[img % 2]
            x_sb = x_sbs[img]

            def make(ch, blk):
                def run():
                    sl = slice(ch * CR * W, (ch + 1) * CR * W)
                    src = x_sb[:, blk, sl].rearrange("p (r c) -> p r c", c=W)
                    quant_stage(src, aq1, blk, 0 + blk, 2 + blk, t1pool,
                                CR, ch * CR)
                return run
            return [make(ch, blk) for ch in range(N_CHUNK) for blk in range(2)]

        def conv1_tile_half(hh, cb):
            # image 0, tile 0 split into 4-row halves: the first half only
            # needs the first 7 quantized rows, starting the PE ~1.5us
            # earlier.  Full-size PSUM tag keeps the pool layout uniform.
            aq1, aq2 = aq1s[0], aq2s[0]
            ps = pspool.tile([128, TQ], F32, tag="ps")
            w4 = w1_sb[:].rearrange("p (t j m) -> p t j m", t=9, j=2)
            for tap in range(9):
                ky, kx = divmod(tap, 3)
                lhsT = w4[:, tap, :, cb * 128:cb * 128 + 128]
                base = 2 * (4 * hh + ky) * PW + kx
                rhs = aq1[:, base:base + 4 * SLOT].rearrange(
                    "p (r j c) -> p j r c", j=2, c=PW)
                nc.tensor.matmul(ps[:, 0:4 * PW], lhsT, rhs,
                                 perf_mode=mybir.MatmulPerfMode.DoubleRow,
                                 start=(tap == 0), stop=(tap == 8))
            psv = ps[:, 0:4 * PW].rearrange("p (r c) -> p r c",
                                            c=PW)[:, :, 0:W]
            quant_stage(psv, aq2, cb, 4 + cb, 6 + cb, t2pool, 4, 4 * hh,
                        tag="q2half")

        def conv1_tile(img, t, cb):
            aq1, aq2 = aq1s[img % 2], aq2s[img % 2]
            ps = conv_tile(aq1, w1_sb, t, cb)
            psv = ps[:].rearrange("p (r c) -> p r c", c=PW)[:, :, 0:W]
            quant_stage(psv, aq2, cb, 4 + cb, 6 + cb, t2pool, RT, t * RT)

        def conv2_tile(img, t, cb):
            aq2, x_sb = aq2s[img % 2], x_sbs[img]
            ps = conv_tile(aq2, w2_sb, t, cb)
            psv = ps[:].rearrange("p (r c) -> p r c", c=PW)[:, :, 0:W]
            res = x_sb[:, cb, t * RT * W: (t + 1) * RT * W]
            res3 = res.rearrange("p (r c) -> p r c", c=W)
            nc.vector.scalar_tensor_tensor(res3, psv, 1.0 / 9.0, res3,
                                           OP.mult, OP.add)
            nc.sync.dma_start(outr[img][:, cb, t * RT * W:
                                        (t + 1) * RT * W], res)

        # Image 0: chunk 0 quantizes in 7-row halves to cut head latency.
        for hh in range(2):
            for blk in range(2):
                src0 = x_sbs[0][:, blk, hh * 7 * W:(hh + 1) * 7 * W].rearrange(
                    "p (r c) -> p r c", c=W)
                quant_stage(src0, aq1s[0], blk, 0 + blk, 2 + blk, t1pool,
                            7, hh * 7, tag="qhalf")
        # Only chunk 1 up front: chunk 2-3 units would head-of-line
        # block the DVE FIFO (their x-DMA lands ~22us) ahead of the q2
        # ops that conv2 t0-t1 wait on; they interleave into the loop.
        u0 = stage1_units(0)
        for f in u0[2:4]:
            f()

        # PE warmup gated on the prm DMA; the PE sequencer itself is not
        # ready before ~10us, so a short burst suffices to prime HAM.
        wu_ps = pspool.tile([128, TQ], F32, tag="ps")
        for i in range(N_WARMUP):
            wu = nc.tensor.matmul(wu_ps[:], wsrc[:, 0:128], wsrc[:, 128:584],
                                  start=(i == 0), stop=(i == N_WARMUP - 1))
            if i == 0:
                tile.add_dep_helper(wu.ins, prm_dma.ins, sync=True,
                                    reason="gate warmup start on prm land")

        # Per-image: conv2 interleaves one tile behind conv1; next image's
        # stage-1 quant units spread between tile groups.
        for img in range(N_IMG):
            if img == 0:
                inter = u0[4:] + stage1_units(1)
            else:
                inter = stage1_units(img + 1) if img + 1 < N_IMG else []
            for t in range(NT):
                if img == 0 and t == 0:
                    for hh in (0, 1):
                        conv1_tile_half(hh, 0)
                        conv1_tile_half(hh, 1)
                else:
                    conv1_tile(img, t, 0)
                    conv1_tile(img, t, 1)
                if t > 1:
                    conv2_tile(img, t - 2, 0)
                    conv2_tile(img, t - 2, 1)
                for _ in range(2):
                    if inter:
                        inter.pop(0)()
            for t in (NT - 2, NT - 1):
                conv2_tile(img, t, 0)
                conv2_tile(img, t, 1)
            for f in inter:
                f()

    nc.compile()
    return nc


def _host_prep(w1, w2, g1, b1, m1, v1, g2, b2, m2, v2):
    """BN folds + DoReFa weight quantization, replicating the reference's
    fp32 op sequence exactly (jax CPU), then weight layout transforms."""
    import jax
    import jax.numpy as jnp
    import ml_dtypes

    cpu = jax.local_devices(backend="cpu")[0]
    with jax.default_device(cpu):
        eps = jnp.float32(1e-5)
        inv1 = g1 / jnp.sqrt(v1 + eps)
        bias1 = b1 - m1 * inv1
        inv2 = g2 / jnp.sqrt(v2 + eps)
        bias2 = b2 - m2 * inv2
        inv2_9 = inv2 / np.float32(9.0)

        def wq3(w):
            wt = jnp.tanh(w)
            wn = wt / (2.0 * jnp.max(jnp.abs(wt))) + 0.5
            return 2.0 * jnp.round(wn * 3.0) - 3.0   # exact ints {-3,-1,1,3}

        wq1 = np.asarray(wq3(jnp.asarray(w1)), dtype=np.float32)
        wq2 = np.asarray(wq3(jnp.asarray(w2)), dtype=np.float32)
        inv1, bias1, inv2_9, bias2 = (
            np.asarray(a, dtype=np.float32)
            for a in (inv1, bias1, inv2_9, bias2))

    def wlayout(wq):
        # [cout, cin, ky, kx] -> [k(128), tap(9), blk(2), cout(256)]
        a = wq.reshape(256, 2, 128, 9)                     # cout, blk, k, tap
        return np.ascontiguousarray(np.transpose(a, (2, 3, 1, 0))
                                    .reshape(128, 4608)
                                    ).astype(ml_dtypes.float8_e4m3)

    prm = np.zeros((128, 8), np.float32)
    for col, v in enumerate((inv1, bias1)):
        prm[:, 2 * col] = v[0:128]
        prm[:, 2 * col + 1] = v[128:256]
    for col, v in enumerate((inv2_9, bias2)):
        prm[:, 4 + 2 * col] = v[0:128]
        prm[:, 4 + 2 * col + 1] = v[128:256]

    return {"w1t": wlayout(wq1), "w2t": wlayout(wq2), "prm": prm}


def kernel(x, w1, w2, g1, b1, m1, v1, g2, b2, m2, v2):
    global LAST_EXEC_NS
    x = np.asarray(x, dtype=np.float32)

    if "nc" not in _CACHED:
        _CACHED["nc"] = _build()
    nc = _CACHED["nc"]

    shared = _host_prep(w1, w2, g1, b1, m1, v1, g2, b2, m2, v2)
    in_maps = []
    for c in range(N_CORES):
        m = dict(shared)
        m["x"] = x[N_IMG * c:N_IMG * (c + 1)]
        in_maps.append(m)

    trace = bool(int(os.environ.get("BASS_TRACE", "0")))
    res = run_bass_kernel_spmd(nc, in_maps, core_ids=list(range(N_CORES)),
                               trace=trace)
    LAST_EXEC_NS = res.exec_time_ns
    return np.concatenate([res.results[c]["out"] for c in range(N_CORES)],
                          axis=0)
